# revision 9
# baseline (speedup 1.0000x reference)
"""Trainium2 Bass kernel for BinaryTreeLatentVariable inside algorithm.

Math (per level d, bottom-up over a complete binary tree in heap order):
    new[pp, n] = p[pp, n] + logsumexp_{i,j}( trans[pp, i, j] + l[i, n] + r[j, n] )

Factorization used here (s[n] = l[0, n] + r[0, n] is a per-node stabilizer):
    new[pp, n] = p[pp, n] + s[n] + log( sum_{ij} expT[ij, pp] * V[ij, n] )
    V[ij, n]   = exp( l[i(ij), n] + r[j(ij), n] - s[n] )     (args stay within ~+-11)
    expT[ij, pp] = exp(trans[...]) permuted to [(lL,lc),(rL,rc)] x [(pL,pc)]

On-device per level (nodes on the free axis, states on partitions):
    - V-args are built by two small PE matmuls with constant 0/1 select
      matrices (K=21 incl. a -1 row that subtracts s, and K=20), into PSUM.
    - ACT exponentiates PSUM -> SBUF.
    - One K=100 PE matmul per 100-row chunk contracts with expT into PSUM.
    - q = p + s (broadcast) is built with two more tiny matmuls (I20, ones).
    - ACT log, DVE add -> level output.

Sharding: 8 trees per core across 8 cores (no cross-core communication).
h is transposed host-side to [D, cols] with columns laid out level-major
(leaves first) so the emission matmul streams in the order the tree phase
consumes, letting the deepest tree level overlap the tail of the h DMA.
"""

import numpy as np

import concourse.bacc as bacc
import concourse.bass as bass
from concourse import mybir, tile
from concourse.bass_utils import run_bass_kernel_spmd

F32 = mybir.dt.float32

B = 64
N_NODES = 1023
D = 512
L = 5
C = 4
LC = L * C          # 20
IJ = 400            # 20 * 20
NCORES = 8
TPC = B // NCORES   # trees per core = 8
DEPTH = 9           # leaves are level 9; internal levels 8..0

# Per-core column layout: level-major blocks (leaves first), t-major inside.
LEVEL_ORDER = list(range(DEPTH, -1, -1))  # 9, 8, ..., 0
OFFS = {}
_off = 0
for _d in LEVEL_ORDER:
    OFFS[_d] = _off
    _off += TPC * (1 << _d)
NCOL = _off  # 8184

COLTILE = 512
SROW = 32           # partition holding the stabilizer row
NROW = 33           # buffer partition count (rows 20..31 stay zero)
KCH = 4           # 400 = 4 x 100 chunks of the ij axis
CHW = IJ // KCH   # 100


def _host_constants(W, b, trans):
    # expT: [400, 20] with row = (lL*4+lc)*20 + (rL*4+rc), col = pL*4+pc,
    # pre-chunked to [100, 4, 20] so SBUF tiles slice on a free dim.
    expT = np.exp(trans.transpose(1, 4, 2, 5, 0, 3).reshape(IJ, LC))
    expT_ch = np.ascontiguousarray(
        expT.reshape(KCH, CHW, LC).transpose(1, 0, 2)).astype(np.float32)

    ij = np.arange(IJ)
    # s lives at partition SROW=32 (DVE/ACT base-partition rule); rows
    # 20..31 of the y/sw buffers are kept zeroed so K=33 matmuls are safe.
    selL = np.zeros((NROW, IJ), np.float32)
    selL[ij // LC, ij] = 1.0
    selL[SROW, :] = -1.0          # subtracts the stabilizer row s
    selR = np.zeros((LC, IJ), np.float32)
    selR[ij % LC, ij] = 1.0

    w_ch = np.ascontiguousarray(W.reshape(KCH, D // KCH, LC)).astype(np.float32)
    b_col = np.ascontiguousarray(b.reshape(LC, 1)).astype(np.float32)
    # q = p + s via one K=33 matmul: rows 0..19 identity (p), row 32 ones (s)
    eye_aug = np.zeros((NROW, LC), np.float32)
    eye_aug[0:LC] = np.eye(LC, dtype=np.float32)
    eye_aug[SROW] = 1.0
    return {
        "expt": expT_ch, "sell": selL, "selr": selR,
        "wch": w_ch, "bcol": b_col, "eyeaug": eye_aug,
    }


def _host_ht(h, core):
    """[512, NCOL] slice for one core: level-major blocks, t-major inside."""
    hk = h[core * TPC:(core + 1) * TPC]          # [8, 1023, 512]
    blocks = []
    for d in LEVEL_ORDER:
        lo, hi = (1 << d) - 1, (1 << (d + 1)) - 1
        blk = hk[:, lo:hi, :]                     # [8, m, 512]
        blocks.append(blk.transpose(2, 0, 1).reshape(D, -1))
    return np.ascontiguousarray(np.concatenate(blocks, axis=1), dtype=np.float32)


def _build_bass():
    nc = bacc.Bacc("TRN2", target_bir_lowering=False)

    ht_d = nc.declare_dram_parameter("ht", [D, NCOL], F32, isOutput=False)
    wch_d = nc.declare_dram_parameter("wch", [KCH, D // KCH, LC], F32, isOutput=False)
    bcol_d = nc.declare_dram_parameter("bcol", [LC, 1], F32, isOutput=False)
    expt_d = nc.declare_dram_parameter("expt", [CHW, KCH, LC], F32, isOutput=False)
    sell_d = nc.declare_dram_parameter("sell", [NROW, IJ], F32, isOutput=False)
    selr_d = nc.declare_dram_parameter("selr", [LC, IJ], F32, isOutput=False)
    eyeaug_d = nc.declare_dram_parameter("eyeaug", [NROW, LC], F32, isOutput=False)
    out_d = nc.declare_dram_parameter("out", [LC, TPC], F32, isOutput=True)

    ID = mybir.ActivationFunctionType.Identity
    EXP = mybir.ActivationFunctionType.Exp
    LN = mybir.ActivationFunctionType.Ln

    with tile.TileContext(nc) as tc:
        with (
            tc.tile_pool(name="consts", bufs=1) as consts,
            tc.tile_pool(name="sw", bufs=1) as swp,
            tc.tile_pool(name="ybufs", bufs=1) as ybp,
            tc.tile_pool(name="ht", bufs=8) as htp,
            tc.tile_pool(name="vtiles", bufs=3) as vtp,
            tc.tile_pool(name="ttiles", bufs=2) as ttp,
            tc.tile_pool(name="ps_sw", bufs=2, space="PSUM") as ps_swp,
            tc.tile_pool(name="ps_exp", bufs=3, space="PSUM") as ps_expp,
            tc.tile_pool(name="ps_out", bufs=2, space="PSUM") as ps_outp,
            tc.tile_pool(name="ps_q", bufs=1, space="PSUM") as ps_qp,
        ):
            # ---- constants into SBUF ----
            w_sb = consts.tile([D // KCH, KCH, LC], F32)
            for k in range(KCH):
                nc.sync.dma_start(w_sb[:, k, :], wch_d[k])
            expt_sb = consts.tile([CHW, KCH, LC], F32)
            nc.sync.dma_start(expt_sb[:], expt_d[:])
            sell_sb = consts.tile([NROW, IJ], F32)
            nc.sync.dma_start(sell_sb[:], sell_d[:])
            selr_sb = consts.tile([LC, IJ], F32)
            nc.sync.dma_start(selr_sb[:], selr_d[:])
            b_sb = consts.tile([LC, 1], F32)
            nc.sync.dma_start(b_sb[:], bcol_d[:])
            eyeaug_sb = consts.tile([NROW, LC], F32)
            nc.sync.dma_start(eyeaug_sb[:], eyeaug_d[:])

            # sw[0:20, c] = emission scores; row 20 holds the stabilizer row
            # for the leaf block.
            sw_sb = swp.tile([NROW, NCOL], F32)
            nc.gpsimd.memset(sw_sb[:], 0.0)

            # ---- phase 1: sw = W^T @ hT + b, streamed in column tiles ----
            n_ct = (NCOL + COLTILE - 1) // COLTILE
            for ct in range(n_ct):
                c0 = ct * COLTILE
                nt = min(COLTILE, NCOL - c0)
                ps = ps_swp.tile([LC, COLTILE], F32, tag="ps_sw")
                for kd in range(KCH):
                    htt = htp.tile([D // KCH, COLTILE], F32, tag="htt")
                    nc.sync.dma_start(
                        htt[:, :nt],
                        ht_d[kd * (D // KCH):(kd + 1) * (D // KCH), c0:c0 + nt])
                    nc.tensor.matmul(
                        ps[:, :nt], w_sb[:, kd, :], htt[:, :nt],
                        start=(kd == 0), stop=(kd == KCH - 1))
                nc.scalar.activation(
                    sw_sb[0:LC, c0:c0 + nt], ps[:, :nt], ID, bias=b_sb[:, 0:1])

            # ---- phase 2: bottom-up tree levels ----
            ybufs = {}
            for d in range(DEPTH - 1, -1, -1):
                n = TPC * (1 << d)
                if d == DEPTH - 1:
                    yprev = sw_sb[:, OFFS[DEPTH]:OFFS[DEPTH] + 2 * n]
                else:
                    yprev = ybufs[d + 1][:]
                ybuf = ybp.tile([NROW, n], F32, tag=f"y{d}", name=f"y{d}")
                nc.gpsimd.memset(ybuf[:], 0.0)
                ybufs[d] = ybuf

                p_off = OFFS[d]
                for c0 in range(0, n, COLTILE):
                    nt = min(COLTILE, n - c0)
                    even = yprev[0:NROW, 2 * c0:2 * (c0 + nt):2]
                    odd = yprev[0:LC, 2 * c0 + 1:2 * (c0 + nt):2]
                    srow = yprev[SROW:SROW + 1, 2 * c0:2 * (c0 + nt):2]

                    # stabilizer s = l0 + r0: spread onto yprev row 20 even
                    # cols (feeds the K=21 select matmul) and compacted into
                    # sw row 20 at the parent's columns (feeds the q matmul).
                    nc.vector.tensor_add(
                        srow,
                        yprev[0:1, 2 * c0:2 * (c0 + nt):2],
                        yprev[0:1, 2 * c0 + 1:2 * (c0 + nt):2])
                    nc.vector.tensor_copy(
                        sw_sb[SROW:SROW + 1, p_off + c0:p_off + c0 + nt], srow)

                    ops_ = ps_outp.tile([LC, COLTILE], F32, tag="ps_out")
                    for kc in range(KCH):
                        eps = ps_expp.tile([CHW, COLTILE], F32, tag="ps_exp")
                        nc.tensor.matmul(
                            eps[:, :nt], sell_sb[:, kc * CHW:(kc + 1) * CHW],
                            even, start=True, stop=False)
                        nc.tensor.matmul(
                            eps[:, :nt], selr_sb[:, kc * CHW:(kc + 1) * CHW],
                            odd, start=False, stop=True)
                        v_sb = vtp.tile([CHW, COLTILE], F32, tag="v")
                        nc.scalar.activation(v_sb[:, :nt], eps[:, :nt], EXP)
                        nc.tensor.matmul(
                            ops_[:, :nt], expt_sb[:, kc, :], v_sb[:, :nt],
                            start=(kc == 0), stop=(kc == KCH - 1))

                    qps = ps_qp.tile([LC, COLTILE], F32, tag="ps_q")
                    nc.tensor.matmul(
                        qps[:, :nt], eyeaug_sb[:],
                        sw_sb[0:NROW, p_off + c0:p_off + c0 + nt],
                        start=True, stop=True)

                    t_sb = ttp.tile([LC, COLTILE], F32, tag="t")
                    nc.scalar.activation(t_sb[:, :nt], ops_[:, :nt], LN)
                    nc.vector.tensor_add(
                        ybuf[0:LC, c0:c0 + nt], t_sb[:, :nt], qps[:, :nt])

            nc.sync.dma_start(out_d[:], ybufs[0][0:LC, 0:TPC])

    nc.compile()
    return nc


_CACHE = {}


def _get_nc():
    if "nc" not in _CACHE:
        _CACHE["nc"] = _build_bass()
    return _CACHE["nc"]


def run(h, W, b, trans, trace=False, **trace_kwargs):
    h = np.ascontiguousarray(np.asarray(h, dtype=np.float32))
    W = np.asarray(W, dtype=np.float32)
    b = np.asarray(b, dtype=np.float32)
    trans = np.asarray(trans, dtype=np.float32)

    consts = _host_constants(W, b, trans)
    in_maps = []
    for core in range(NCORES):
        m = dict(consts)
        m["ht"] = _host_ht(h, core)
        in_maps.append(m)

    nc = _get_nc()
    res = run_bass_kernel_spmd(nc, in_maps, list(range(NCORES)),
                               trace=trace, **trace_kwargs)
    outs = [res.results[k]["out"] for k in range(NCORES)]  # each [20, 8]
    full = np.concatenate([o.T for o in outs], axis=0).reshape(B, L, C)
    return np.ascontiguousarray(full.astype(np.float32)), res


def kernel(h, W, b, trans):
    out, _ = run(h, W, b, trans, trace=False)
    return out


# revision 12
# speedup vs baseline: 2.9087x; 2.9087x over previous
"""Trainium2 Bass kernel for BinaryTreeLatentVariable inside algorithm.

Math (per level d, bottom-up over a complete binary tree in heap order):
    new[pp, n] = p[pp, n] + logsumexp_{i,j}( trans[pp, i, j] + l[i, n] + r[j, n] )

Factorization (s[n] = l[0, n] + r[0, n]):
    new[pp, n] = p[pp, n] + s[n] + log( sum_{ij} expT[ij, pp] * V[ij, n] )
    V[ij, n]   = exp( lnorm[i, n] + rnorm[j, n] ),  xnorm[i] = x[i] - x[0]
    expT       = exp(trans) permuted to [(lL,lc),(rL,rc)] x [(pL,pc)]

Representation: levels are stored NORMALIZED (per node, relative to state 0)
and DEINTERLEAVED (sibling pairs share a column: left child in partitions
0..19, right child in partitions 64..83), in bf16 — normalized values stay
within ~+-12 so bf16 matmuls are safe.  The state-0 absolute values flow
through a separate fp32 "z" row: zsum[n] = l[0,n] + r[0,n] per parent.

Per level tile (nodes on the free axis):
    - 4x select matmul (K=84, bf16 const 0/1 matrix): args = lnorm_i + rnorm_j
    - ACT exp (PSUM -> SBUF bf16), 4x contraction matmul with expT (K=100)
    - ACT ln of the accumulated sums, DVE u = ln + p (emission)
    - normalization matmul (K=20, I - e0 column pattern), DVE deinterleave
    - GPSIMD carries the fp32 z-chain (zrow = u0 + zsum; next zsum = pairs)

Phase 1 feeds the tree: emission sw = W^T @ hT + b via PE (h cast to bf16
host-side, halving HBM traffic; columns laid out level-major, leaves first,
so the deepest level overlaps the tail of the h DMA).

Sharding: 8 trees per core across 8 cores (no cross-core communication).
"""

import ml_dtypes
import numpy as np

import concourse.bacc as bacc
import concourse.bass as bass
from concourse import mybir, tile
from concourse.bass_utils import run_bass_kernel_spmd

F32 = mybir.dt.float32
BF16 = mybir.dt.bfloat16
NP_BF16 = ml_dtypes.bfloat16

B = 64
N_NODES = 1023
D = 512
L = 5
C = 4
LC = L * C          # 20
IJ = 400            # 20 * 20
NCORES = 8
TPC = B // NCORES   # trees per core = 8
DEPTH = 9           # leaves are level 9; internal levels 8..0

# Per-core column layout: level-major blocks (leaves first), t-major inside.
LEVEL_ORDER = list(range(DEPTH, -1, -1))  # 9, 8, ..., 0
OFFS = {}
_off = 0
for _d in LEVEL_ORDER:
    OFFS[_d] = _off
    _off += TPC * (1 << _d)
NCOL = _off                      # 8184
NLEAFC = TPC * (1 << DEPTH)      # 4096 leaf columns
NCOLI = NCOL - NLEAFC            # 4088 internal columns
OFFSI = {d: OFFS[d] - NLEAFC for d in range(DEPTH)}

COLTILE = 512
DMATILE = 1024
ROWR = 64           # partition base of the right-child (odd) block
NROWY = 84          # ybuf partitions: 0..19 left, 64..83 right, rest zero
KCH = 4             # 400 = 4 x 100 chunks of the ij axis
CHW = IJ // KCH     # 100


def _host_constants(W, b, trans):
    # expT: [400, 20], row = (lL*4+lc)*20 + (rL*4+rc), col = pL*4+pc,
    # chunked to [100, 4, 20] so SBUF tiles slice on a free dim.
    expT = np.exp(trans.astype(np.float64).transpose(1, 4, 2, 5, 0, 3)
                  .reshape(IJ, LC))
    expT_ch = np.ascontiguousarray(
        expT.reshape(KCH, CHW, LC).transpose(1, 0, 2)).astype(NP_BF16)

    ij = np.arange(IJ)
    selLR = np.zeros((NROWY, IJ), NP_BF16)
    selLR[ij // LC, ij] = 1.0            # left-child state select
    selLR[ROWR + ij % LC, ij] = 1.0      # right-child state select

    w_ch = np.ascontiguousarray(
        W.reshape(KCH, D // KCH, LC)).astype(NP_BF16)
    b_col = np.ascontiguousarray(b.reshape(LC, 1)).astype(np.float32)

    # normalization: out[i] = u[i] - u[0]  (column 0 of lhsT is zero)
    normmat = np.zeros((LC, LC), NP_BF16)
    for i in range(1, LC):
        normmat[i, i] = 1.0
        normmat[0, i] = -1.0
    ones_row = np.ones((1, LC), np.float32)
    return {
        "expt": expT_ch, "sellr": selLR, "wch": w_ch, "bcol": b_col,
        "normmat": normmat, "onesr": ones_row,
    }


def _host_ht(h, core):
    """bf16 [512, NCOL] slice for one core: level-major, t-major inside."""
    hk = h[core * TPC:(core + 1) * TPC]          # [8, 1023, 512]
    blocks = []
    for d in LEVEL_ORDER:
        lo, hi = (1 << d) - 1, (1 << (d + 1)) - 1
        blk = hk[:, lo:hi, :]                     # [8, m, 512]
        blocks.append(blk.transpose(2, 0, 1).reshape(D, -1))
    out = np.concatenate(blocks, axis=1)
    return np.ascontiguousarray(out).astype(NP_BF16)


def _patch_act_tables(nc):
    """Retarget every activation-table load to natural_log_exp_and_others
    (covers Exp, Ln and Identity) and drop the now-redundant reloads, which
    otherwise cost ~1.3us each when Exp and Ln alternate."""
    from concourse.hw_specs import get_activation_tables
    tables = list(get_activation_tables(nc.m.arch).items())
    target = None
    for idx, (name, _fns) in enumerate(tables):
        if name == "natural_log_exp_and_others":
            target = idx
    if target is None:
        return
    for fn in nc.m.functions:
        kept = False
        for blk in fn.blocks:
            new_insts = []
            for ins in blk.instructions:
                if isinstance(ins, mybir.InstLoadActFuncSet):
                    si = ins.sync_info
                    has_sems = si is not None and (
                        len(si.on_wait) > 0 or len(si.on_update) > 0)
                    if not kept or has_sems:
                        ins.act_func_set_id = target
                        kept = True
                        new_insts.append(ins)
                    continue
                new_insts.append(ins)
            blk.instructions[:] = new_insts


def _build_bass():
    nc = bacc.Bacc("TRN2", target_bir_lowering=False)

    ht_d = nc.declare_dram_parameter("ht", [D, NCOL], BF16, isOutput=False)
    wch_d = nc.declare_dram_parameter("wch", [KCH, D // KCH, LC], BF16,
                                      isOutput=False)
    bcol_d = nc.declare_dram_parameter("bcol", [LC, 1], F32, isOutput=False)
    expt_d = nc.declare_dram_parameter("expt", [CHW, KCH, LC], BF16,
                                       isOutput=False)
    sellr_d = nc.declare_dram_parameter("sellr", [NROWY, IJ], BF16,
                                        isOutput=False)
    normmat_d = nc.declare_dram_parameter("normmat", [LC, LC], BF16,
                                          isOutput=False)
    onesr_d = nc.declare_dram_parameter("onesr", [1, LC], F32, isOutput=False)
    out_d = nc.declare_dram_parameter("out", [LC, TPC], F32, isOutput=True)

    ID = mybir.ActivationFunctionType.Identity
    EXP = mybir.ActivationFunctionType.Exp
    LN = mybir.ActivationFunctionType.Ln

    with tile.TileContext(nc) as tc:
        with (
            tc.tile_pool(name="consts", bufs=1) as consts,
            tc.tile_pool(name="sw", bufs=1) as swp,
            tc.tile_pool(name="ybufs", bufs=1) as ybp,
            tc.tile_pool(name="ht", bufs=8) as htp,
            tc.tile_pool(name="vtiles", bufs=3) as vtp,
            tc.tile_pool(name="utiles", bufs=3) as utp,
            tc.tile_pool(name="ttiles", bufs=2) as ttp,
            tc.tile_pool(name="ps_sw", bufs=2, space="PSUM") as ps_swp,
            tc.tile_pool(name="ps_exp", bufs=2, space="PSUM") as ps_expp,
            tc.tile_pool(name="ps_out", bufs=2, space="PSUM") as ps_outp,
            tc.tile_pool(name="ps_norm", bufs=2, space="PSUM") as ps_normp,
        ):
            # ---- constants ----
            w_sb = consts.tile([D // KCH, KCH, LC], BF16)
            for k in range(KCH):
                nc.sync.dma_start(w_sb[:, k, :], wch_d[k])
            expt_sb = consts.tile([CHW, KCH, LC], BF16)
            nc.sync.dma_start(expt_sb[:], expt_d[:])
            sellr_sb = consts.tile([NROWY, IJ], BF16)
            nc.sync.dma_start(sellr_sb[:], sellr_d[:])
            b_sb = consts.tile([LC, 1], F32)
            nc.sync.dma_start(b_sb[:], bcol_d[:])
            normmat_sb = consts.tile([LC, LC], BF16)
            nc.sync.dma_start(normmat_sb[:], normmat_d[:])
            onesr_sb = consts.tile([1, LC], F32)
            nc.sync.dma_start(onesr_sb[:], onesr_d[:])

            # emissions of internal nodes (p), fp32
            sw_sb = swp.tile([LC, NCOLI], F32)
            # z rows (base partition 0 each): zsum = s per parent; zrow scratch
            zsum_sb = swp.tile([1, NCOLI], F32)
            zrow_sb = swp.tile([1, NCOLI], F32)

            # per-level normalized deinterleaved buffers, bf16
            ybufs = {}
            for d in range(DEPTH, 0, -1):
                npair = TPC * (1 << d) // 2
                yb = ybp.tile([NROWY, npair], BF16, tag=f"y{d}", name=f"y{d}")
                nc.gpsimd.memset(yb[:], 0.0)
                ybufs[d] = yb

            # ---- phase 1: sw / leaf-level from hT ----
            for ct in range(0, NCOL, DMATILE):
                dw = min(DMATILE, NCOL - ct)
                htts = []
                for kd in range(KCH):
                    htt = htp.tile([D // KCH, DMATILE], BF16, tag="htt",
                                   name="htt")
                    nc.sync.dma_start(
                        htt[:, :dw],
                        ht_d[kd * (D // KCH):(kd + 1) * (D // KCH),
                             ct:ct + dw])
                    htts.append(htt)
                for half in range(0, dw, COLTILE):
                    nt = min(COLTILE, dw - half)
                    c0 = ct + half
                    ps = ps_swp.tile([LC, COLTILE], F32, tag="ps_sw",
                                     name="ps_sw")
                    for kd in range(KCH):
                        nc.tensor.matmul(
                            ps[:, :nt], w_sb[:, kd, :],
                            htts[kd][:, half:half + nt],
                            start=(kd == 0), stop=(kd == KCH - 1))
                    if c0 < NLEAFC:
                        # leaf columns -> normalized y9 + zsum for level 8
                        u_sb = utp.tile([LC, COLTILE], BF16, tag="u",
                                        name="u")
                        nc.scalar.activation(u_sb[:, :nt], ps[:, :nt], ID,
                                             bias=b_sb[:, 0:1])
                        pn = ps_normp.tile([LC, COLTILE], F32, tag="ps_norm",
                                           name="ps_norm")
                        nc.tensor.matmul(pn[:, :nt], normmat_sb[:],
                                         u_sb[:, :nt], start=True, stop=True)
                        pair0 = c0 // 2
                        nh = nt // 2
                        y9 = ybufs[DEPTH]
                        nc.vector.tensor_copy(
                            y9[0:LC, pair0:pair0 + nh], pn[:, 0:nt:2])
                        nc.vector.tensor_copy(
                            y9[ROWR:ROWR + LC, pair0:pair0 + nh],
                            pn[:, 1:nt:2])
                        nc.gpsimd.tensor_add(
                            zsum_sb[0:1, pair0:pair0 + nh],
                            u_sb[0:1, 0:nt:2], u_sb[0:1, 1:nt:2])
                    else:
                        nc.scalar.activation(
                            sw_sb[0:LC, c0 - NLEAFC:c0 - NLEAFC + nt],
                            ps[:, :nt], ID, bias=b_sb[:, 0:1])

            # ---- phase 2: bottom-up tree levels ----
            for d in range(DEPTH - 1, -1, -1):
                n = TPC * (1 << d)
                yprev = ybufs[d + 1]
                p_off = OFFSI[d]
                for c0 in range(0, n, COLTILE):
                    nt = min(COLTILE, n - c0)
                    ops_ = ps_outp.tile([LC, COLTILE], F32, tag="ps_out",
                                        name="ps_out")
                    for kc in range(KCH):
                        eps = ps_expp.tile([CHW, COLTILE], F32, tag="ps_exp",
                                           name="ps_exp")
                        nc.tensor.matmul(
                            eps[:, :nt],
                            sellr_sb[:, kc * CHW:(kc + 1) * CHW],
                            yprev[0:NROWY, c0:c0 + nt],
                            start=True, stop=True)
                        v_sb = vtp.tile([CHW, COLTILE], BF16, tag="v",
                                        name="v")
                        nc.scalar.activation(v_sb[:, :nt], eps[:, :nt], EXP)
                        nc.tensor.matmul(
                            ops_[:, :nt], expt_sb[:, kc, :], v_sb[:, :nt],
                            start=(kc == 0), stop=(kc == KCH - 1))

                    t_sb = ttp.tile([LC, COLTILE], F32, tag="t", name="t")
                    nc.scalar.activation(t_sb[:, :nt], ops_[:, :nt], LN)

                    if d == 0:
                        # final: Y = t + p + zsum broadcast
                        qps = ps_normp.tile([LC, COLTILE], F32,
                                            tag="ps_norm", name="ps_norm")
                        nc.tensor.matmul(qps[:, :nt], onesr_sb[:],
                                         zsum_sb[0:1, p_off:p_off + nt],
                                         start=True, stop=True)
                        y0a = utp.tile([LC, TPC], F32, tag="y0a", name="y0a")
                        nc.vector.tensor_add(
                            y0a[:], t_sb[:, :nt], sw_sb[0:LC,
                                                        p_off:p_off + nt])
                        y0b = utp.tile([LC, TPC], F32, tag="y0b", name="y0b")
                        nc.vector.tensor_add(y0b[:], y0a[:], qps[:, :nt])
                        nc.sync.dma_start(out_d[:], y0b[:])
                        continue

                    # u = t + p (bf16 for the normalization matmul)
                    u_sb = utp.tile([LC, COLTILE], BF16, tag="u", name="u")
                    nc.vector.tensor_add(
                        u_sb[:, :nt], t_sb[:, :nt],
                        sw_sb[0:LC, p_off + c0:p_off + c0 + nt])
                    pn = ps_normp.tile([LC, COLTILE], F32, tag="ps_norm",
                                       name="ps_norm")
                    nc.tensor.matmul(pn[:, :nt], normmat_sb[:], u_sb[:, :nt],
                                     start=True, stop=True)
                    pair0 = c0 // 2
                    nh = nt // 2
                    yb = ybufs[d]
                    nc.vector.tensor_copy(
                        yb[0:LC, pair0:pair0 + nh], pn[:, 0:nt:2])
                    nc.vector.tensor_copy(
                        yb[ROWR:ROWR + LC, pair0:pair0 + nh], pn[:, 1:nt:2])
                    # z-chain (fp32, on GPSIMD): zrow = t0 + p0 + zsum
                    nc.gpsimd.tensor_add(
                        zrow_sb[0:1, p_off + c0:p_off + c0 + nt],
                        t_sb[0:1, :nt],
                        sw_sb[0:1, p_off + c0:p_off + c0 + nt])
                    nc.gpsimd.tensor_add(
                        zrow_sb[0:1, p_off + c0:p_off + c0 + nt],
                        zrow_sb[0:1, p_off + c0:p_off + c0 + nt],
                        zsum_sb[0:1, p_off + c0:p_off + c0 + nt])
                    # next level's zsum = pairwise sums of zrow
                    nc.gpsimd.tensor_add(
                        zsum_sb[0:1, OFFSI[d - 1] + pair0:
                                OFFSI[d - 1] + pair0 + nh],
                        zrow_sb[0:1, p_off + c0:p_off + c0 + nt:2],
                        zrow_sb[0:1, p_off + c0 + 1:p_off + c0 + nt:2])

    nc.compile()
    _patch_act_tables(nc)
    return nc


_CACHE = {}


def _get_nc():
    if "nc" not in _CACHE:
        _CACHE["nc"] = _build_bass()
    return _CACHE["nc"]


def run(h, W, b, trans, trace=False, **trace_kwargs):
    h = np.asarray(h, dtype=np.float32)
    W = np.asarray(W, dtype=np.float32)
    b = np.asarray(b, dtype=np.float32)
    trans = np.asarray(trans, dtype=np.float32)

    consts = _host_constants(W, b, trans)
    in_maps = []
    for core in range(NCORES):
        m = dict(consts)
        m["ht"] = _host_ht(h, core)
        in_maps.append(m)

    nc = _get_nc()
    res = run_bass_kernel_spmd(nc, in_maps, list(range(NCORES)),
                               trace=trace, **trace_kwargs)
    outs = [res.results[k]["out"] for k in range(NCORES)]  # each [20, 8]
    full = np.concatenate([np.asarray(o, np.float32).T for o in outs],
                          axis=0).reshape(B, L, C)
    return np.ascontiguousarray(full), res


def kernel(h, W, b, trans):
    out, _ = run(h, W, b, trans, trace=False)
    return out


# revision 14
# speedup vs baseline: 3.2742x; 1.1256x over previous
"""Trainium2 Bass kernel for BinaryTreeLatentVariable inside algorithm.

Math (per level d, bottom-up over a complete binary tree in heap order):
    new[pp, n] = p[pp, n] + logsumexp_{i,j}( trans[pp, i, j] + l[i, n] + r[j, n] )

Factorization (s[n] = l[0, n] + r[0, n]):
    new[pp, n] = p[pp, n] + s[n] + log( sum_{ij} expT[ij, pp] * V[ij, n] )
    V[ij, n]   = exp( lnorm[i, n] + rnorm[j, n] ),  xnorm[i] = x[i] - x[0]
    expT       = exp(trans) permuted to [(lL,lc),(rL,rc)] x [(pL,pc)]

Representation: levels are stored NORMALIZED (relative to each node's
state-0 score, so values stay within ~+-12 and bf16 matmuls are safe) and
DEINTERLEAVED (sibling pairs share a column: left child in partitions
0..19, right child in partitions 64..83; state-0 rows carry don't-care
absolute values — the select matrices have zero weight there because
xnorm[0] == 0).  Absolute state-0 scores flow through a separate fp32
z-chain: zsum[n] = l[0,n] + r[0,n] per parent node.

Per level tile (nodes on the free axis):
    - 4x select matmul (K=84, bf16 0/1 matrix): args = lnorm_i + rnorm_j,
      written into paired PSUM banks so one ACT exp covers two chunks
    - 2x ACT exp (PSUM pair -> SBUF bf16), 4x contraction matmul with expT
    - ACT ln of the accumulated sums, DVE u = ln + p, normalization matmul
      (K=20, columns e_i - e_0; column 0 = e_0 keeps the absolute row),
      DVE deinterleave into the next ybuf
    - GPSIMD carries the fp32 z-chain (zrow = ln0 + (p0 + zsum); pairs)

Phase 1: emission sw = W^T @ hT + b on PE; h is cast to bf16 host-side
(halving HBM traffic) and laid out level-major (leaves first) so the
deepest level overlaps the tail of the h DMA.  Leaf columns use
host-normalized weights Wn (column i -> W_i - W_0, except column 0) so
leaf outputs drop straight into the normalized representation.

Sharding: 8 trees per core across 8 cores (no cross-core communication).
"""

import ml_dtypes
import numpy as np

import concourse.bacc as bacc
import concourse.bass as bass
from concourse import mybir, tile
from concourse.bass_utils import run_bass_kernel_spmd

F32 = mybir.dt.float32
BF16 = mybir.dt.bfloat16
NP_BF16 = ml_dtypes.bfloat16

B = 64
N_NODES = 1023
D = 512
L = 5
C = 4
LC = L * C          # 20
IJ = 400            # 20 * 20
NCORES = 8
TPC = B // NCORES   # trees per core = 8
DEPTH = 9           # leaves are level 9; internal levels 8..0

# Per-core column layout: level-major blocks (leaves first), t-major inside.
LEVEL_ORDER = list(range(DEPTH, -1, -1))  # 9, 8, ..., 0
OFFS = {}
_off = 0
for _d in LEVEL_ORDER:
    OFFS[_d] = _off
    _off += TPC * (1 << _d)
NCOL = _off                      # 8184
NLEAFC = TPC * (1 << DEPTH)      # 4096 leaf columns
NCOLI = NCOL - NLEAFC            # 4088 internal columns
OFFSI = {d: OFFS[d] - NLEAFC for d in range(DEPTH)}

COLTILE = 512
DMATILE = 2048
ROWR = 64           # partition base of the right-child (odd) block
NROWY = 84          # ybuf partitions: 0..19 left, 64..83 right, rest zero
KCH = 4             # 400 = 4 x 100 chunks of the ij axis
CHW = IJ // KCH     # 100


def _host_constants(W, b, trans):
    # expT: [400, 20], row = (lL*4+lc)*20 + (rL*4+rc), col = pL*4+pc,
    # chunked to [100, 4, 20] so SBUF tiles slice on a free dim.
    expT = np.exp(trans.astype(np.float64).transpose(1, 4, 2, 5, 0, 3)
                  .reshape(IJ, LC))
    expT_ch = np.ascontiguousarray(
        expT.reshape(KCH, CHW, LC).transpose(1, 0, 2)).astype(NP_BF16)

    ij = np.arange(IJ)
    selLR = np.zeros((NROWY, IJ), NP_BF16)
    selLR[ij // LC, ij] = 1.0            # left-child state select
    selLR[ROWR + ij % LC, ij] = 1.0      # right-child state select
    selLR[0, :] = 0.0                    # lnorm[0] == 0: ignore row 0
    selLR[ROWR, :] = 0.0                 # rnorm[0] == 0: ignore row 64

    # weights: internal columns use W; leaf columns use the normalized
    # Wn (col i -> W_i - W_0 for i>0) so leaf psums are already normalized
    # with the absolute state-0 score kept in row 0.
    Wn = W - W[:, 0:1]
    Wn[:, 0] = W[:, 0]
    wboth = np.concatenate(
        [W.reshape(KCH, D // KCH, LC), Wn.reshape(KCH, D // KCH, LC)],
        axis=0)                                   # [8, 128, 20]
    w_both = np.ascontiguousarray(wboth).astype(NP_BF16)

    bn = b - b[0]
    bn[0] = b[0]
    bcols = np.stack([b, bn], axis=1).astype(np.float32)   # [20, 2]

    # normalization: col i>0 -> u_i - u_0; col 0 -> u_0 (absolute kept)
    normmat = np.zeros((LC, LC), NP_BF16)
    normmat[0, 0] = 1.0
    for i in range(1, LC):
        normmat[i, i] = 1.0
        normmat[0, i] = -1.0
    ones_row = np.ones((1, LC), np.float32)
    return {
        "expt": expT_ch, "sellr": selLR, "wboth": w_both, "bcols": bcols,
        "normmat": normmat, "onesr": ones_row,
    }


def _host_ht(h, core):
    """bf16 [512, NCOL] slice for one core: level-major, t-major inside."""
    hk = h[core * TPC:(core + 1) * TPC]          # [8, 1023, 512]
    blocks = []
    for d in LEVEL_ORDER:
        lo, hi = (1 << d) - 1, (1 << (d + 1)) - 1
        blk = hk[:, lo:hi, :]                     # [8, m, 512]
        blocks.append(blk.transpose(2, 0, 1).reshape(D, -1))
    out = np.concatenate(blocks, axis=1)
    return np.ascontiguousarray(out).astype(NP_BF16)


def _patch_act_tables(nc):
    """Retarget every activation-table load to natural_log_exp_and_others
    (covers Exp, Ln and Identity) and drop the now-redundant reloads, which
    otherwise cost ~1.3us each when Exp and Ln alternate."""
    from concourse.hw_specs import get_activation_tables
    tables = list(get_activation_tables(nc.m.arch).items())
    target = None
    for idx, (name, _fns) in enumerate(tables):
        if name == "natural_log_exp_and_others":
            target = idx
    if target is None:
        return
    for fn in nc.m.functions:
        kept = False
        for blk in fn.blocks:
            new_insts = []
            for ins in blk.instructions:
                if isinstance(ins, mybir.InstLoadActFuncSet):
                    si = ins.sync_info
                    has_sems = si is not None and (
                        len(si.on_wait) > 0 or len(si.on_update) > 0)
                    if not kept or has_sems:
                        ins.act_func_set_id = target
                        kept = True
                        new_insts.append(ins)
                    continue
                new_insts.append(ins)
            blk.instructions[:] = new_insts


def _build_bass():
    nc = bacc.Bacc("TRN2", target_bir_lowering=False)

    ht_d = nc.declare_dram_parameter("ht", [D, NCOL], BF16, isOutput=False)
    wboth_d = nc.declare_dram_parameter("wboth", [2 * KCH, D // KCH, LC],
                                        BF16, isOutput=False)
    bcols_d = nc.declare_dram_parameter("bcols", [LC, 2], F32, isOutput=False)
    expt_d = nc.declare_dram_parameter("expt", [CHW, KCH, LC], BF16,
                                       isOutput=False)
    sellr_d = nc.declare_dram_parameter("sellr", [NROWY, IJ], BF16,
                                        isOutput=False)
    normmat_d = nc.declare_dram_parameter("normmat", [LC, LC], BF16,
                                          isOutput=False)
    onesr_d = nc.declare_dram_parameter("onesr", [1, LC], F32, isOutput=False)
    out_d = nc.declare_dram_parameter("out", [LC, TPC], F32, isOutput=True)

    EXP = mybir.ActivationFunctionType.Exp
    LN = mybir.ActivationFunctionType.Ln
    ADD = mybir.AluOpType.add

    with tile.TileContext(nc) as tc:
        with (
            tc.tile_pool(name="consts", bufs=1) as consts,
            tc.tile_pool(name="sw", bufs=1) as swp,
            tc.tile_pool(name="ybufs", bufs=1) as ybp,
            tc.tile_pool(name="ht", bufs=8) as htp,
            tc.tile_pool(name="vtiles", bufs=4) as vtp,
            tc.tile_pool(name="utiles", bufs=3) as utp,
            tc.tile_pool(name="ttiles", bufs=2) as ttp,
            tc.tile_pool(name="ps_sw", bufs=2, space="PSUM") as ps_swp,
            tc.tile_pool(name="ps_exp", bufs=2, space="PSUM") as ps_expp,
            tc.tile_pool(name="ps_out", bufs=1, space="PSUM") as ps_outp,
            tc.tile_pool(name="ps_norm", bufs=1, space="PSUM") as ps_normp,
        ):
            # ---- constants (issued on the GPSIMD queue so the SP queue
            # starts streaming hT immediately) ----
            w_sb = consts.tile([D // KCH, 2 * KCH, LC], BF16)
            nc.gpsimd.dma_start(w_sb[:], wboth_d[:].transpose([1, 0, 2]))
            expt_sb = consts.tile([CHW, KCH, LC], BF16)
            nc.gpsimd.dma_start(expt_sb[:], expt_d[:])
            sellr_sb = consts.tile([NROWY, IJ], BF16)
            nc.gpsimd.dma_start(sellr_sb[:], sellr_d[:])
            b_sb = consts.tile([LC, 2], F32)
            nc.gpsimd.dma_start(b_sb[:], bcols_d[:])
            normmat_sb = consts.tile([LC, LC], BF16)
            nc.gpsimd.dma_start(normmat_sb[:], normmat_d[:])
            onesr_sb = consts.tile([1, LC], F32)
            nc.gpsimd.dma_start(onesr_sb[:], onesr_d[:])

            # emissions of internal nodes (p), fp32
            sw_sb = swp.tile([LC, NCOLI], F32)
            # fp32 z rows: zsum = s per parent; pz = p0 + zsum; zrow scratch
            zsum_sb = swp.tile([1, NCOLI], F32)
            pz_sb = swp.tile([1, NCOLI], F32)
            zrow_sb = swp.tile([1, NCOLI], F32)

            # per-level normalized deinterleaved buffers, bf16
            ybufs = {}
            for d in range(DEPTH, 0, -1):
                npair = TPC * (1 << d) // 2
                yb = ybp.tile([NROWY, npair], BF16, tag=f"y{d}", name=f"y{d}")
                nc.gpsimd.memset(yb[:], 0.0)
                ybufs[d] = yb

            # ---- phase 1: sw / leaf level from hT ----
            for ct in range(0, NCOL, DMATILE):
                dw = min(DMATILE, NCOL - ct)
                htts = []
                for kd in range(KCH):
                    htt = htp.tile([D // KCH, DMATILE], BF16, tag="htt",
                                   name="htt")
                    nc.sync.dma_start(
                        htt[:, :dw],
                        ht_d[kd * (D // KCH):(kd + 1) * (D // KCH),
                             ct:ct + dw])
                    htts.append(htt)
                for half in range(0, dw, COLTILE):
                    nt = min(COLTILE, dw - half)
                    c0 = ct + half
                    leaf = c0 < NLEAFC
                    wof = KCH if leaf else 0
                    ps = ps_swp.tile([LC, COLTILE], F32, tag="ps_sw",
                                     name="ps_sw")
                    for kd in range(KCH):
                        nc.tensor.matmul(
                            ps[:, :nt], w_sb[:, wof + kd, :],
                            htts[kd][:, half:half + nt],
                            start=(kd == 0), stop=(kd == KCH - 1))
                    if leaf:
                        # normalized already; deinterleave + bias into y9
                        pair0 = c0 // 2
                        nh = nt // 2
                        y9 = ybufs[DEPTH]
                        nc.vector.tensor_scalar(
                            y9[0:LC, pair0:pair0 + nh], ps[:, 0:nt:2],
                            b_sb[:, 1:2], None, ADD)
                        nc.vector.tensor_scalar(
                            y9[ROWR:ROWR + LC, pair0:pair0 + nh],
                            ps[:, 1:nt:2], b_sb[:, 1:2], None, ADD)
                        # zsum for level 8 = abs left + abs right (state-0
                        # rows).  Walrus requires equal base partitions for
                        # SB+SB tensor_tensor, so stage both rows at
                        # partition 0 (pz/zrow regions are free until the
                        # level-8 pass and cover the same columns).
                        nc.vector.tensor_scalar(
                            pz_sb[0:1, pair0:pair0 + nh], ps[0:1, 0:nt:2],
                            b_sb[0:1, 1:2], None, ADD)
                        nc.vector.tensor_scalar(
                            zrow_sb[0:1, pair0:pair0 + nh], ps[0:1, 1:nt:2],
                            b_sb[0:1, 1:2], None, ADD)
                        nc.gpsimd.tensor_add(
                            zsum_sb[0:1, pair0:pair0 + nh],
                            pz_sb[0:1, pair0:pair0 + nh],
                            zrow_sb[0:1, pair0:pair0 + nh])
                    else:
                        nc.vector.tensor_scalar(
                            sw_sb[0:LC, c0 - NLEAFC:c0 - NLEAFC + nt],
                            ps[:, :nt], b_sb[:, 0:1], None, ADD)

            # ---- phase 2: bottom-up tree levels ----
            for d in range(DEPTH - 1, -1, -1):
                n = TPC * (1 << d)
                yprev = ybufs[d + 1]
                p_off = OFFSI[d]
                if d > 0:
                    # pz = p0 + zsum, ready before this level's ln lands
                    nc.gpsimd.tensor_add(
                        pz_sb[0:1, p_off:p_off + n],
                        sw_sb[0:1, p_off:p_off + n],
                        zsum_sb[0:1, p_off:p_off + n])
                for c0 in range(0, n, COLTILE):
                    nt = min(COLTILE, n - c0)
                    ops_ = ps_outp.tile([LC, COLTILE], F32, tag="ps_out",
                                        name="ps_out")
                    for kp in range(KCH // 2):
                        eps = ps_expp.tile([CHW, 2, COLTILE], F32,
                                           tag="ps_exp", name="ps_exp")
                        for kk in range(2):
                            kc = 2 * kp + kk
                            nc.tensor.matmul(
                                eps[:, kk, :nt],
                                sellr_sb[:, kc * CHW:(kc + 1) * CHW],
                                yprev[0:NROWY, c0:c0 + nt],
                                start=True, stop=True)
                        v_sb = vtp.tile([CHW, 2, COLTILE], BF16, tag="v",
                                        name="v")
                        nc.scalar.activation(v_sb[:, :, :nt],
                                             eps[:, :, :nt], EXP)
                        for kk in range(2):
                            kc = 2 * kp + kk
                            nc.tensor.matmul(
                                ops_[:, :nt], expt_sb[:, kc, :],
                                v_sb[:, kk, :nt],
                                start=(kc == 0), stop=(kc == KCH - 1))

                    t_sb = ttp.tile([LC, COLTILE], F32, tag="t", name="t")
                    nc.scalar.activation(t_sb[:, :nt], ops_[:, :nt], LN)

                    if d == 0:
                        # final: Y = t + p + zsum broadcast
                        qps = ps_normp.tile([LC, COLTILE], F32,
                                            tag="ps_norm", name="ps_norm")
                        nc.tensor.matmul(qps[:, :nt], onesr_sb[:],
                                         zsum_sb[0:1, p_off:p_off + nt],
                                         start=True, stop=True)
                        y0a = utp.tile([LC, TPC], F32, tag="y0a", name="y0a")
                        nc.vector.tensor_add(
                            y0a[:], t_sb[:, :nt],
                            sw_sb[0:LC, p_off:p_off + nt])
                        y0b = utp.tile([LC, TPC], F32, tag="y0b", name="y0b")
                        nc.vector.tensor_add(y0b[:], y0a[:], qps[:, :nt])
                        nc.sync.dma_start(out_d[:], y0b[:])
                        continue

                    # u = t + p (bf16), normalize, deinterleave into ybuf
                    u_sb = utp.tile([LC, COLTILE], BF16, tag="u", name="u")
                    nc.vector.tensor_add(
                        u_sb[:, :nt], t_sb[:, :nt],
                        sw_sb[0:LC, p_off + c0:p_off + c0 + nt])
                    pn = ps_normp.tile([LC, COLTILE], F32, tag="ps_norm",
                                       name="ps_norm")
                    nc.tensor.matmul(pn[:, :nt], normmat_sb[:], u_sb[:, :nt],
                                     start=True, stop=True)
                    pair0 = c0 // 2
                    nh = nt // 2
                    yb = ybufs[d]
                    nc.vector.tensor_copy(
                        yb[0:LC, pair0:pair0 + nh], pn[:, 0:nt:2])
                    nc.vector.tensor_copy(
                        yb[ROWR:ROWR + LC, pair0:pair0 + nh], pn[:, 1:nt:2])
                    # fp32 z-chain on GPSIMD: zrow = ln0 + (p0 + zsum)
                    nc.gpsimd.tensor_add(
                        zrow_sb[0:1, p_off + c0:p_off + c0 + nt],
                        t_sb[0:1, :nt],
                        pz_sb[0:1, p_off + c0:p_off + c0 + nt])
                    nc.gpsimd.tensor_add(
                        zsum_sb[0:1, OFFSI[d - 1] + pair0:
                                OFFSI[d - 1] + pair0 + nh],
                        zrow_sb[0:1, p_off + c0:p_off + c0 + nt:2],
                        zrow_sb[0:1, p_off + c0 + 1:p_off + c0 + nt:2])

    nc.compile()
    _patch_act_tables(nc)
    return nc


_CACHE = {}


def _get_nc():
    if "nc" not in _CACHE:
        _CACHE["nc"] = _build_bass()
    return _CACHE["nc"]


def run(h, W, b, trans, trace=False, **trace_kwargs):
    h = np.asarray(h, dtype=np.float32)
    W = np.asarray(W, dtype=np.float32)
    b = np.asarray(b, dtype=np.float32)
    trans = np.asarray(trans, dtype=np.float32)

    consts = _host_constants(W, b, trans)
    in_maps = []
    for core in range(NCORES):
        m = dict(consts)
        m["ht"] = _host_ht(h, core)
        in_maps.append(m)

    nc = _get_nc()
    res = run_bass_kernel_spmd(nc, in_maps, list(range(NCORES)),
                               trace=trace, **trace_kwargs)
    outs = [res.results[k]["out"] for k in range(NCORES)]  # each [20, 8]
    full = np.concatenate([np.asarray(o, np.float32).T for o in outs],
                          axis=0).reshape(B, L, C)
    return np.ascontiguousarray(full), res


def kernel(h, W, b, trans):
    out, _ = run(h, W, b, trans, trace=False)
    return out


# revision 15
# speedup vs baseline: 3.6057x; 1.1013x over previous
"""Trainium2 Bass kernel for BinaryTreeLatentVariable inside algorithm.

Math (per level d, bottom-up over a complete binary tree in heap order):
    new[pp, n] = p[pp, n] + logsumexp_{i,j}( trans[pp, i, j] + l[i, n] + r[j, n] )

Factorization (s[n] = l[0, n] + r[0, n]):
    new[pp, n] = p[pp, n] + s[n] + log( sum_{ij} expT[ij, pp] * V[ij, n] )
    V[ij, n]   = exp( lnorm[i, n] + rnorm[j, n] ),  xnorm[i] = x[i] - x[0]
    expT       = exp(trans) permuted to [(lL,lc),(rL,rc)] x [(pL,pc)]

Representation: levels are stored NORMALIZED (relative to each node's
state-0 score, so values stay within ~+-12 and bf16 matmuls are safe) and
DEINTERLEAVED (sibling pairs share a column: left child in partitions
0..19, right child in partitions 64..83; state-0 rows carry don't-care
absolute values — the select matrices have zero weight there because
xnorm[0] == 0).  Absolute state-0 scores flow through a separate fp32
z-chain: zsum[n] = l[0,n] + r[0,n] per parent node.

Per level tile (nodes on the free axis):
    - 4x select matmul (K=84, bf16 0/1 matrix): args = lnorm_i + rnorm_j,
      written into paired PSUM banks so one ACT exp covers two chunks
    - 2x ACT exp (PSUM pair -> SBUF bf16), 4x contraction matmul with expT
    - ACT ln of the accumulated sums, DVE u = ln + p, normalization matmul
      (K=20, columns e_i - e_0; column 0 = e_0 keeps the absolute row),
      DVE deinterleave into the next ybuf
    - GPSIMD carries the fp32 z-chain (zrow = ln0 + (p0 + zsum); pairs)

Phase 1: emission sw = W^T @ hT + b on PE; h is cast to bf16 host-side
(halving HBM traffic) and laid out level-major (leaves first) so the
deepest level overlaps the tail of the h DMA.  Leaf columns use
host-normalized weights Wn (column i -> W_i - W_0, except column 0) so
leaf outputs drop straight into the normalized representation.

Sharding: 8 trees per core across 8 cores (no cross-core communication).
"""

import ml_dtypes
import numpy as np

import concourse.bacc as bacc
import concourse.bass as bass
from concourse import mybir, tile
from concourse.bass_utils import run_bass_kernel_spmd

F32 = mybir.dt.float32
BF16 = mybir.dt.bfloat16
NP_BF16 = ml_dtypes.bfloat16

B = 64
N_NODES = 1023
D = 512
L = 5
C = 4
LC = L * C          # 20
IJ = 400            # 20 * 20
NCORES = 8
TPC = B // NCORES   # trees per core = 8
DEPTH = 9           # leaves are level 9; internal levels 8..0

# Per-core column layout: level-major blocks (leaves first), t-major inside.
LEVEL_ORDER = list(range(DEPTH, -1, -1))  # 9, 8, ..., 0
OFFS = {}
_off = 0
for _d in LEVEL_ORDER:
    OFFS[_d] = _off
    _off += TPC * (1 << _d)
NCOL = _off                      # 8184
NLEAFC = TPC * (1 << DEPTH)      # 4096 leaf columns
NCOLI = NCOL - NLEAFC            # 4088 internal columns
OFFSI = {d: OFFS[d] - NLEAFC for d in range(DEPTH)}

COLTILE = 512
DMATILE = 2048
ROWR = 64           # partition base of the right-child (odd) block
NROWY = 84          # ybuf partitions: 0..19 left, 64..83 right, rest zero
KCH = 4             # 400 = 4 x 100 chunks of the ij axis
CHW = IJ // KCH     # 100


def _host_constants(W, b, trans):
    # expT: [400, 20], row = (lL*4+lc)*20 + (rL*4+rc), col = pL*4+pc,
    # chunked to [100, 4, 20] so SBUF tiles slice on a free dim.
    expT = np.exp(trans.astype(np.float64).transpose(1, 4, 2, 5, 0, 3)
                  .reshape(IJ, LC))
    expT_ch = np.ascontiguousarray(
        expT.reshape(KCH, CHW, LC).transpose(1, 0, 2)).astype(NP_BF16)

    ij = np.arange(IJ)
    selLR = np.zeros((NROWY, IJ), NP_BF16)
    selLR[ij // LC, ij] = 1.0            # left-child state select
    selLR[ROWR + ij % LC, ij] = 1.0      # right-child state select
    selLR[0, :] = 0.0                    # lnorm[0] == 0: ignore row 0
    selLR[ROWR, :] = 0.0                 # rnorm[0] == 0: ignore row 64

    # normalized weights everywhere: col i -> W_i - W_0 for i>0; col 0
    # keeps W_0 so row 0 of every emission is the absolute state-0 score.
    Wn = W - W[:, 0:1]
    Wn[:, 0] = W[:, 0]
    w_both = np.ascontiguousarray(
        Wn.reshape(KCH, D // KCH, LC)).astype(NP_BF16)

    bn = (b - b[0]).astype(np.float32)
    bn[0] = b[0]
    bcols = bn.reshape(LC, 1)

    # normalization: col i>0 -> u_i - u_0; col 0 -> u_0 (absolute kept)
    normmat = np.zeros((LC, LC), NP_BF16)
    normmat[0, 0] = 1.0
    for i in range(1, LC):
        normmat[i, i] = 1.0
        normmat[0, i] = -1.0
    ones_row = np.ones((1, LC), np.float32)
    return {
        "expt": expT_ch, "sellr": selLR, "wboth": w_both, "bcols": bcols,
        "normmat": normmat, "onesr": ones_row,
    }


def _host_ht(h, core):
    """bf16 [512, NCOL] slice for one core: level-major, t-major inside."""
    hk = h[core * TPC:(core + 1) * TPC]          # [8, 1023, 512]
    blocks = []
    for d in LEVEL_ORDER:
        lo, hi = (1 << d) - 1, (1 << (d + 1)) - 1
        blk = hk[:, lo:hi, :]                     # [8, m, 512]
        blocks.append(blk.transpose(2, 0, 1).reshape(D, -1))
    out = np.concatenate(blocks, axis=1)
    return np.ascontiguousarray(out).astype(NP_BF16)


def _patch_act_tables(nc):
    """Retarget every activation-table load to natural_log_exp_and_others
    (covers Exp, Ln and Identity) and drop the now-redundant reloads, which
    otherwise cost ~1.3us each when Exp and Ln alternate."""
    from concourse.hw_specs import get_activation_tables
    tables = list(get_activation_tables(nc.m.arch).items())
    target = None
    for idx, (name, _fns) in enumerate(tables):
        if name == "natural_log_exp_and_others":
            target = idx
    if target is None:
        return
    for fn in nc.m.functions:
        kept = False
        for blk in fn.blocks:
            new_insts = []
            for ins in blk.instructions:
                if isinstance(ins, mybir.InstLoadActFuncSet):
                    si = ins.sync_info
                    has_sems = si is not None and (
                        len(si.on_wait) > 0 or len(si.on_update) > 0)
                    if not kept or has_sems:
                        ins.act_func_set_id = target
                        kept = True
                        new_insts.append(ins)
                    continue
                new_insts.append(ins)
            blk.instructions[:] = new_insts


def _build_bass():
    nc = bacc.Bacc("TRN2", target_bir_lowering=False)

    ht_d = nc.declare_dram_parameter("ht", [D, NCOL], BF16, isOutput=False)
    wboth_d = nc.declare_dram_parameter("wboth", [KCH, D // KCH, LC],
                                        BF16, isOutput=False)
    bcols_d = nc.declare_dram_parameter("bcols", [LC, 1], F32, isOutput=False)
    expt_d = nc.declare_dram_parameter("expt", [CHW, KCH, LC], BF16,
                                       isOutput=False)
    sellr_d = nc.declare_dram_parameter("sellr", [NROWY, IJ], BF16,
                                        isOutput=False)
    normmat_d = nc.declare_dram_parameter("normmat", [LC, LC], BF16,
                                          isOutput=False)
    onesr_d = nc.declare_dram_parameter("onesr", [1, LC], F32, isOutput=False)
    out_d = nc.declare_dram_parameter("out", [LC, TPC], F32, isOutput=True)

    EXP = mybir.ActivationFunctionType.Exp
    LN = mybir.ActivationFunctionType.Ln
    ADD = mybir.AluOpType.add

    with tile.TileContext(nc) as tc:
        with (
            tc.tile_pool(name="consts", bufs=1) as consts,
            tc.tile_pool(name="sw", bufs=1) as swp,
            tc.tile_pool(name="ybufs", bufs=1) as ybp,
            tc.tile_pool(name="ht", bufs=8) as htp,
            tc.tile_pool(name="vtiles", bufs=4) as vtp,
            tc.tile_pool(name="utiles", bufs=3) as utp,
            tc.tile_pool(name="ttiles", bufs=2) as ttp,
            tc.tile_pool(name="ps_sw", bufs=2, space="PSUM") as ps_swp,
            tc.tile_pool(name="ps_exp", bufs=2, space="PSUM") as ps_expp,
            tc.tile_pool(name="ps_out", bufs=1, space="PSUM") as ps_outp,
            tc.tile_pool(name="ps_norm", bufs=1, space="PSUM") as ps_normp,
        ):
            # ---- constants (issued on the GPSIMD queue so the SP queue
            # starts streaming hT immediately) ----
            w_sb = consts.tile([D // KCH, KCH, LC], BF16)
            nc.gpsimd.dma_start(w_sb[:], wboth_d[:].transpose([1, 0, 2]))
            expt_sb = consts.tile([CHW, KCH, LC], BF16)
            nc.gpsimd.dma_start(expt_sb[:], expt_d[:])
            sellr_sb = consts.tile([NROWY, IJ], BF16)
            nc.gpsimd.dma_start(sellr_sb[:], sellr_d[:])
            b_sb = consts.tile([LC, 1], F32)
            nc.gpsimd.dma_start(b_sb[:], bcols_d[:])
            normmat_sb = consts.tile([LC, LC], BF16)
            nc.gpsimd.dma_start(normmat_sb[:], normmat_d[:])
            onesr_sb = consts.tile([1, LC], F32)
            nc.gpsimd.dma_start(onesr_sb[:], onesr_d[:])

            # normalized emissions of internal nodes (row 0 = absolute p0)
            sw_sb = swp.tile([LC, NCOLI], F32)
            # per-tree accumulator of absolute state-0 scores (z-total)
            zacc = swp.tile([1, TPC], F32)
            zfin = swp.tile([1, TPC], F32)
            nc.vector.memset(zacc[:], 0.0)

            # per-level normalized deinterleaved buffers, bf16
            ybufs = {}
            for d in range(DEPTH, 0, -1):
                npair = TPC * (1 << d) // 2
                yb = ybp.tile([NROWY, npair], BF16, tag=f"y{d}", name=f"y{d}")
                nc.gpsimd.memset(yb[:], 0.0)
                ybufs[d] = yb

            # ---- phase 1: sw / leaf level from hT ----
            for ct in range(0, NCOL, DMATILE):
                dw = min(DMATILE, NCOL - ct)
                htts = []
                for kd in range(KCH):
                    htt = htp.tile([D // KCH, DMATILE], BF16, tag="htt",
                                   name="htt")
                    nc.sync.dma_start(
                        htt[:, :dw],
                        ht_d[kd * (D // KCH):(kd + 1) * (D // KCH),
                             ct:ct + dw])
                    htts.append(htt)
                for half in range(0, dw, COLTILE):
                    nt = min(COLTILE, dw - half)
                    c0 = ct + half
                    leaf = c0 < NLEAFC
                    ps = ps_swp.tile([LC, COLTILE], F32, tag="ps_sw",
                                     name="ps_sw")
                    for kd in range(KCH):
                        nc.tensor.matmul(
                            ps[:, :nt], w_sb[:, kd, :],
                            htts[kd][:, half:half + nt],
                            start=(kd == 0), stop=(kd == KCH - 1))
                    if leaf:
                        # normalized already; deinterleave + bias into y9
                        pair0 = c0 // 2
                        nh = nt // 2
                        y9 = ybufs[DEPTH]
                        nc.vector.tensor_scalar(
                            y9[0:LC, pair0:pair0 + nh], ps[:, 0:nt:2],
                            b_sb[:, 0:1], None, ADD)
                        nc.vector.tensor_scalar(
                            y9[ROWR:ROWR + LC, pair0:pair0 + nh],
                            ps[:, 1:nt:2], b_sb[:, 0:1], None, ADD)
                    else:
                        nc.vector.tensor_scalar(
                            sw_sb[0:LC, c0 - NLEAFC:c0 - NLEAFC + nt],
                            ps[:, :nt], b_sb[:, 0:1], None, ADD)

            # ---- phase 2: bottom-up tree levels ----
            for d in range(DEPTH - 1, -1, -1):
                n = TPC * (1 << d)
                yprev = ybufs[d + 1]
                p_off = OFFSI[d]
                # fold the completed child level's absolute state-0 scores
                # (ybuf rows 0 and 64) into the per-tree z accumulator
                npair_pt = (1 << d)          # pairs per tree in yprev
                for row in (0, ROWR):
                    rsum = utp.tile([1, TPC], F32, tag="rsum", name="rsum")
                    nc.vector.tensor_reduce(
                        rsum[:], yprev[row:row + 1, :].rearrange(
                            "p (t q) -> p t q", t=TPC),
                        mybir.AxisListType.X, ADD)
                    nc.vector.tensor_add(zacc[:], zacc[:], rsum[:])
                for c0 in range(0, n, COLTILE):
                    nt = min(COLTILE, n - c0)
                    ops_ = ps_outp.tile([LC, COLTILE], F32, tag="ps_out",
                                        name="ps_out")
                    for kp in range(KCH // 2):
                        eps = ps_expp.tile([CHW, 2, COLTILE], F32,
                                           tag="ps_exp", name="ps_exp")
                        for kk in range(2):
                            kc = 2 * kp + kk
                            nc.tensor.matmul(
                                eps[:, kk, :nt],
                                sellr_sb[:, kc * CHW:(kc + 1) * CHW],
                                yprev[0:NROWY, c0:c0 + nt],
                                start=True, stop=True)
                        v_sb = vtp.tile([CHW, 2, COLTILE], BF16, tag="v",
                                        name="v")
                        nc.scalar.activation(v_sb[:, :, :nt],
                                             eps[:, :, :nt], EXP)
                        for kk in range(2):
                            kc = 2 * kp + kk
                            nc.tensor.matmul(
                                ops_[:, :nt], expt_sb[:, kc, :],
                                v_sb[:, kk, :nt],
                                start=(kc == 0), stop=(kc == KCH - 1))

                    tdt = F32 if d == 0 else BF16
                    t_sb = ttp.tile([LC, COLTILE], tdt, tag=f"t{tdt}",
                                    name="t")
                    nc.scalar.activation(t_sb[:, :nt], ops_[:, :nt], LN)

                    if d == 0:
                        # final: Y = t + p_norm + (ztotal + p0_root) bcast,
                        # then undo the double-counted p0 on row 0.
                        nc.vector.tensor_add(
                            zfin[:], zacc[:],
                            sw_sb[0:1, p_off:p_off + nt])
                        qps = ps_normp.tile([LC, COLTILE], F32,
                                            tag="ps_norm", name="ps_norm")
                        nc.tensor.matmul(qps[:, :nt], onesr_sb[:],
                                         zfin[:], start=True, stop=True)
                        y0a = utp.tile([LC, TPC], F32, tag="y0a", name="y0a")
                        nc.vector.tensor_add(
                            y0a[:], t_sb[:, :nt],
                            sw_sb[0:LC, p_off:p_off + nt])
                        y0b = utp.tile([LC, TPC], F32, tag="y0b", name="y0b")
                        nc.vector.tensor_add(y0b[:], y0a[:], qps[:, :nt])
                        nc.vector.tensor_sub(
                            y0b[0:1, :], y0b[0:1, :],
                            sw_sb[0:1, p_off:p_off + nt])
                        nc.sync.dma_start(out_d[:], y0b[:])
                        continue

                    # normalize ln-scores, add normalized p, deinterleave
                    pn = ps_normp.tile([LC, COLTILE], F32, tag="ps_norm",
                                       name="ps_norm")
                    nc.tensor.matmul(pn[:, :nt], normmat_sb[:],
                                     t_sb[:, :nt], start=True, stop=True)
                    pair0 = c0 // 2
                    nh = nt // 2
                    yb = ybufs[d]
                    nc.vector.tensor_add(
                        yb[0:LC, pair0:pair0 + nh], pn[:, 0:nt:2],
                        sw_sb[0:LC, p_off + c0:p_off + c0 + nt:2])
                    nc.vector.tensor_add(
                        yb[ROWR:ROWR + LC, pair0:pair0 + nh], pn[:, 1:nt:2],
                        sw_sb[0:LC, p_off + c0 + 1:p_off + c0 + nt:2])

    nc.compile()
    _patch_act_tables(nc)
    return nc


_CACHE = {}


def _get_nc():
    if "nc" not in _CACHE:
        _CACHE["nc"] = _build_bass()
    return _CACHE["nc"]


def run(h, W, b, trans, trace=False, **trace_kwargs):
    h = np.asarray(h, dtype=np.float32)
    W = np.asarray(W, dtype=np.float32)
    b = np.asarray(b, dtype=np.float32)
    trans = np.asarray(trans, dtype=np.float32)

    consts = _host_constants(W, b, trans)
    in_maps = []
    for core in range(NCORES):
        m = dict(consts)
        m["ht"] = _host_ht(h, core)
        in_maps.append(m)

    nc = _get_nc()
    res = run_bass_kernel_spmd(nc, in_maps, list(range(NCORES)),
                               trace=trace, **trace_kwargs)
    outs = [res.results[k]["out"] for k in range(NCORES)]  # each [20, 8]
    full = np.concatenate([np.asarray(o, np.float32).T for o in outs],
                          axis=0).reshape(B, L, C)
    return np.ascontiguousarray(full), res


def kernel(h, W, b, trans):
    out, _ = run(h, W, b, trans, trace=False)
    return out
